# revision 29
# baseline (speedup 1.0000x reference)
"""DiffGRUCell fused kernel for Trainium2 (Bass/Tile), 8-core data-parallel.

Computes, for x = reshape(diffused_x, (B*N, K*F)) and h = h_prev:
    z = sigmoid([x, h] @ Wz + bz)
    r = sigmoid([x, h] @ Wr + br)
    c = tanh([x, r*h] @ Wc + bc)
    h_new = (1 - z) * h + z * c

Sharding: data-parallel over batch. B*N = 20800 tokens are split into 8
contiguous shards of 2600 tokens (8 batches each); gate weights are
replicated. No collectives needed.

Per-core layout strategy:
  - Activations are transposed on-chip (TensorE + identity) into
    feature-major tiles actT[j] = combined.T tile of the 1216-row
    contraction dim, h-part first: j=0..7 -> h[128j:128(j+1)],
    j=8 -> x[0:128], j=9 -> x[128:192] + bias-ones row.
  - Gate biases are folded into the GEMM: weight c-tile j=9 carries the
    bias as an extra row, matched by a constant-1.0 row in actT[9].
  - r is computed feature-major ([h_tile, tokens], weights stationary) so
    rh = sigmoid(r) * h.T is immediately usable as the stationary operand
    of the candidate GEMM.
  - z and c are computed token-major ([tokens, h], activations stationary,
    weights moving) so the final elementwise combine with the
    naturally-loaded h_prev and the output store need no transposes.
"""

import numpy as np

from concourse import bacc
import concourse.mybir as mybir
from concourse.tile import TileContext
from concourse.masks import make_identity
from concourse.bass_utils import run_bass_kernel_spmd

B, N, K, F, H = 64, 325, 3, 64, 1024
XW = K * F            # 192
CONCAT = XW + H       # 1216
NCORES = 8
TPC = (B * N) // NCORES   # 2600 tokens per core
TB = 256                  # token block size
F32 = mybir.dt.float32
MM_DT = mybir.dt.float32r  # matmul compute dtype (float32r: 4x faster PE)


def build(tpc=TPC, tb=TB, mm_dt=MM_DT):
    nc = bacc.Bacc("TRN2")
    x = nc.declare_dram_parameter("x", [tpc, XW], F32, isOutput=False)
    h = nc.declare_dram_parameter("h", [tpc, H], F32, isOutput=False)
    W = {}
    bvec = {}
    for g in "zrc":
        W[g] = nc.declare_dram_parameter(f"W{g}", [CONCAT, H], F32, isOutput=False)
        bvec[g] = nc.declare_dram_parameter(f"b{g}", [H], F32, isOutput=False)
    out = nc.declare_dram_parameter("out", [tpc, H], F32, isOutput=True)

    SIG = mybir.ActivationFunctionType.Sigmoid
    TANH = mybir.ActivationFunctionType.Tanh

    with TileContext(nc) as tc:
        with (
            tc.tile_pool(name="wpool", bufs=1) as wpool,
            tc.tile_pool(name="cpool", bufs=1) as cpool,
            tc.tile_pool(name="xnat", bufs=6) as xpool,
            tc.tile_pool(name="hnat", bufs=5) as hpool,
            tc.tile_pool(name="actT", bufs=20) as apool,
            tc.tile_pool(name="rh", bufs=8) as rhpool,
            tc.tile_pool(name="zsb", bufs=4) as zpool,
            tc.tile_pool(name="csb", bufs=3) as cbpool,
            tc.tile_pool(name="rps", bufs=2, space="PSUM") as rps,
            tc.tile_pool(name="zcps", bufs=6, space="PSUM") as zcps,
        ):
            # Block table: full tb-token blocks, with the sub-128 remainder
            # merged into the final block (a tiny trailing block would pay
            # full LDWEIGHTS cost in its r GEMMs for almost no work).
            nblocks = tpc // tb
            rem = tpc - nblocks * tb
            btbs = [tb] * nblocks
            if rem >= 128 or nblocks == 0:
                btbs.append(rem)
            elif rem:
                btbs[-1] += rem
            blk = []
            t0 = 0
            for btb in btbs:
                blk.append((t0, btb))
                t0 += btb
            max_btb = max(b for _, b in blk)
            order = list(range(len(blk)))

            idt = cpool.tile([128, 128], F32, tag="idt")
            make_identity(nc, idt)
            ones = cpool.tile([1, max_btb], F32, tag="ones")
            nc.vector.memset(ones, 1.0)


            def emit_loads(bidx, chunked=False):
                t0, btb = blk[bidx]
                nsub = (btb + 127) // 128
                xts = []
                hts = []
                for s in range(nsub):
                    r0 = t0 + s * 128
                    ts_ = min(128, t0 + btb - r0)
                    # h before x: the transposes consume the h-part first
                    ht = hpool.tile([128, H], F32, tag="hnat", name=f"hn{bidx}_{s}")
                    if chunked:
                        # halve the first transfers so the first transposes
                        # can start ~1.5us earlier at kernel start
                        nc.sync.dma_start(
                            out=ht[:ts_, 0:512], in_=h[r0 : r0 + ts_, 0:512]
                        )
                        nc.sync.dma_start(
                            out=ht[:ts_, 512:H], in_=h[r0 : r0 + ts_, 512:H]
                        )
                    else:
                        nc.sync.dma_start(out=ht[:ts_, :], in_=h[r0 : r0 + ts_, :])
                    xt = xpool.tile([128, XW], F32, tag="xnat", name=f"xn{bidx}_{s}")
                    nc.sync.dma_start(out=xt[:ts_, :], in_=x[r0 : r0 + ts_, :])
                    xts.append((xt, ts_))
                    hts.append((ht, ts_))
                return xts, hts

            def emit_transposes(bidx, xts, hts):
                t0, btb = blk[bidx]
                nsub = (btb + 127) // 128
                a = [
                    apool.tile([128, max_btb], mm_dt, tag="actT", name=f"actT{bidx}_{i}")
                    for i in range(10)
                ]
                nc.scalar.copy(out=a[9][64:65, :btb], in_=ones[:, :btb])
                for s in range(nsub):
                    xt, ts_ = xts[s]
                    ht, _ = hts[s]
                    srcs = [(ht, 128 * k, 128, a[k]) for k in range(8)]
                    srcs += [(xt, 0, 128, a[8]), (xt, 128, 64, a[9])]
                    for src, c0, cw, dst in srcs:
                        pt = rps.tile([128, 128], F32, tag="rps")
                        nc.tensor.transpose(
                            pt[:cw, :ts_], src[:ts_, c0 : c0 + cw], idt[:ts_, :ts_]
                        )
                        nc.vector.tensor_copy(
                            out=dst[0:cw, s * 128 : s * 128 + ts_], in_=pt[:cw, :ts_]
                        )
                return a

            # Prologue: first two blocks' activations load + transpose before
            # any GEMM, so the PE has work while the 15MB of weights stream in.
            def emit_weights(g):
                # h-part tiles first (contraction runs h-first), then x-part
                # and the bias row folded into the final 65-row tile.
                tiles = []
                for k in range(8):
                    t = wpool.tile([128, H], mm_dt, tag=f"w{g}h{k}")
                    nc.sync.dma_start(
                        out=t,
                        in_=W[g][XW + 128 * k : XW + 128 * (k + 1), :].bitcast(mm_dt),
                    )
                    tiles.append((t, 128))
                t = wpool.tile([128, H], mm_dt, tag=f"w{g}0")
                nc.sync.dma_start(out=t, in_=W[g][0:128, :].bitcast(mm_dt))
                tiles.append((t, 128))
                t = wpool.tile([128, H], mm_dt, tag=f"w{g}1")
                nc.sync.dma_start(out=t[0:64, :], in_=W[g][128:192, :].bitcast(mm_dt))
                nc.sync.dma_start(
                    out=t[64:65, :], in_=bvec[g][:].unsqueeze(0).bitcast(mm_dt)
                )
                tiles.append((t, 65))
                return tiles

            # DMA priority order: first two blocks' activations, then the
            # weights in gate-use order (z warms the PE clock gate first).
            state = {}
            wt = {}
            state[order[0]] = emit_loads(order[0], chunked=True)
            if len(order) > 1:
                state[order[1]] = emit_loads(order[1], chunked=True)
            wt["z"] = emit_weights("z")
            wt["r"] = emit_weights("r")
            wt["c"] = emit_weights("c")

            acts = {}
            acts[order[0]] = emit_transposes(order[0], *state[order[0]])
            if len(order) > 1:
                acts[order[1]] = emit_transposes(order[1], *state[order[1]])

            def emit_r(bidx, a):
                btb = blk[bidx][1]
                rh = [
                    rhpool.tile([128, max_btb], mm_dt, tag="rh", name=f"rh{bidx}_{i}")
                    for i in range(8)
                ]
                for k in range(8):
                    pr = rps.tile([128, max_btb], F32, tag="rps")
                    for j, (wtile, kk) in enumerate(wt["r"]):
                        nc.tensor.matmul(
                            pr[:, :btb],
                            lhsT=wtile[:kk, 128 * k : 128 * (k + 1)],
                            rhs=a[j][:kk, :btb],
                            start=(j == 0),
                            stop=(j == 9),
                        )
                    nc.scalar.activation(out=rh[k][:, :btb], in_=pr[:, :btb], func=SIG)
                    nc.vector.tensor_mul(
                        rh[k][:, :btb], rh[k][:, :btb], a[k][:128, :btb]
                    )
                return rh

            def emit_z(bidx, a, xts):
                btb = blk[bidx][1]
                nsub = (btb + 127) // 128
                zts = []
                for s in range(nsub):
                    _, ts_ = xts[s]
                    zt = zpool.tile([128, H], F32, tag="zsb", name=f"z{bidx}_{s}")
                    for hh in range(2):
                        pz = zcps.tile([128, 512], F32, tag="zcps")
                        for j, (wtile, kk) in enumerate(wt["z"]):
                            nc.tensor.matmul(
                                pz[:ts_, :],
                                lhsT=a[j][:kk, s * 128 : s * 128 + ts_],
                                rhs=wtile[:kk, 512 * hh : 512 * (hh + 1)],
                                start=(j == 0),
                                stop=(j == 9),
                            )
                        nc.scalar.activation(
                            out=zt[:ts_, 512 * hh : 512 * (hh + 1)],
                            in_=pz[:ts_, :],
                            func=SIG,
                        )
                    zts.append(zt)
                return zts

            def emit_c(bidx, a, rh, zts, hts):
                t0, btb = blk[bidx]
                nsub = (btb + 127) // 128
                for s in range(nsub):
                    ht, ts_ = hts[s]
                    ct = cbpool.tile([128, H], F32, tag="csb", name=f"c{bidx}_{s}")
                    for hh in range(2):
                        pc = zcps.tile([128, 512], F32, tag="zcps")
                        for j, (wtile, kk) in enumerate(wt["c"]):
                            lhs_src = rh[j] if j < 8 else a[j]
                            nc.tensor.matmul(
                                pc[:ts_, :],
                                lhsT=lhs_src[:kk, s * 128 : s * 128 + ts_],
                                rhs=wtile[:kk, 512 * hh : 512 * (hh + 1)],
                                start=(j == 0),
                                stop=(j == 9),
                            )
                        nc.scalar.activation(
                            out=ct[:ts_, 512 * hh : 512 * (hh + 1)],
                            in_=pc[:ts_, :],
                            func=TANH,
                        )
                    # h_new = h + z*(c - h), computed in place in ct
                    r0 = t0 + s * 128
                    nc.vector.tensor_sub(ct[:ts_, :], ct[:ts_, :], ht[:ts_, :])
                    nc.vector.tensor_mul(ct[:ts_, :], ct[:ts_, :], zts[s][:ts_, :])
                    nc.vector.tensor_add(ct[:ts_, :], ct[:ts_, :], ht[:ts_, :])
                    nc.sync.dma_start(out=out[r0 : r0 + ts_, :], in_=ct[:ts_, :])

            if len(order) >= 2:
                # Startup interleave: z1 fills the PE while Wc still streams
                # in, so c0 never exposes a weight-arrival stall.
                b0, b1 = order[0], order[1]
                z0 = emit_z(b0, acts[b0], state[b0][0])
                rh0 = emit_r(b0, acts[b0])
                if len(order) > 2:
                    state[order[2]] = emit_loads(order[2])
                z1 = emit_z(b1, acts[b1], state[b1][0])
                emit_c(b0, acts[b0], rh0, z0, state[b0][1])
                if len(order) > 2:
                    acts[order[2]] = emit_transposes(order[2], *state[order[2]])
                if len(order) > 3:
                    state[order[3]] = emit_loads(order[3])
                rh1 = emit_r(b1, acts[b1])
                emit_c(b1, acts[b1], rh1, z1, state[b1][1])
                acts.pop(b0)
                acts.pop(b1)
                if len(order) > 3:
                    acts[order[3]] = emit_transposes(order[3], *state[order[3]])
                start_i = 2
            else:
                start_i = 0

            for i in range(start_i, len(order)):
                bidx = order[i]
                if i + 2 < len(order):
                    state[order[i + 2]] = emit_loads(order[i + 2])
                a = acts.pop(bidx)
                zts = emit_z(bidx, a, state[bidx][0])
                rh = emit_r(bidx, a)
                emit_c(bidx, a, rh, zts, state[bidx][1])
                if i + 2 < len(order):
                    acts[order[i + 2]] = emit_transposes(
                        order[i + 2], *state[order[i + 2]]
                    )

    nc.finalize()
    return nc


_NC_CACHE = {}


def _get_nc():
    key = (TPC, TB, str(MM_DT))
    if key not in _NC_CACHE:
        _NC_CACHE[key] = build()
    return _NC_CACHE[key]


def _make_in_maps(diffused_x, h_prev, Wz, bz, Wr, br, Wc, bc, tpc=TPC):
    x = np.ascontiguousarray(
        np.asarray(diffused_x, dtype=np.float32).reshape(B * N, XW)
    )
    hp = np.ascontiguousarray(np.asarray(h_prev, dtype=np.float32).reshape(B * N, H))
    shared = {
        "Wz": Wz, "bz": bz, "Wr": Wr, "br": br, "Wc": Wc, "bc": bc,
    }
    shared = {
        k: np.ascontiguousarray(np.asarray(v, dtype=np.float32))
        for k, v in shared.items()
    }
    in_maps = []
    for c in range(NCORES):
        sl = slice(c * tpc, (c + 1) * tpc)
        m = {"x": x[sl], "h": hp[sl]}
        m.update(shared)
        in_maps.append(m)
    return in_maps


def kernel(diffused_x, h_prev, Wz, bz, Wr, br, Wc, bc):
    nc = _get_nc()
    in_maps = _make_in_maps(diffused_x, h_prev, Wz, bz, Wr, br, Wc, bc)
    res = run_bass_kernel_spmd(nc, in_maps, list(range(NCORES)))
    outs = [res.results[c]["out"] for c in range(NCORES)]
    return np.concatenate(outs, axis=0).reshape(B, N, H)


def kernel_traced(diffused_x, h_prev, Wz, bz, Wr, br, Wc, bc):
    """Like kernel() but with NTFF profiling; returns (out, BassKernelResults)."""
    nc = _get_nc()
    in_maps = _make_in_maps(diffused_x, h_prev, Wz, bz, Wr, br, Wc, bc)
    res = run_bass_kernel_spmd(nc, in_maps, list(range(NCORES)), trace=True)
    outs = [res.results[c]["out"] for c in range(NCORES)]
    return np.concatenate(outs, axis=0).reshape(B, N, H), res


# revision 31
# speedup vs baseline: 1.0405x; 1.0405x over previous
"""DiffGRUCell fused kernel for Trainium2 (Bass/Tile), 8-core data-parallel.

Computes, for x = reshape(diffused_x, (B*N, K*F)) and h = h_prev:
    z = sigmoid([x, h] @ Wz + bz)
    r = sigmoid([x, h] @ Wr + br)
    c = tanh([x, r*h] @ Wc + bc)
    h_new = (1 - z) * h + z * c

Sharding: data-parallel over batch. B*N = 20800 tokens are split into 8
contiguous shards of 2600 tokens (8 batches each); gate weights are
replicated. No collectives needed.

Per-core layout strategy:
  - Activations are transposed on-chip (TensorE + identity) into
    feature-major tiles actT[j] = combined.T tile of the 1216-row
    contraction dim, h-part first: j=0..7 -> h[128j:128(j+1)],
    j=8 -> x[0:128], j=9 -> x[128:192] + bias-ones row.
  - Gate biases are folded into the GEMM: weight c-tile j=9 carries the
    bias as an extra row, matched by a constant-1.0 row in actT[9].
  - r is computed feature-major ([h_tile, tokens], weights stationary) so
    rh = sigmoid(r) * h.T is immediately usable as the stationary operand
    of the candidate GEMM.
  - z and c are computed token-major ([tokens, h], activations stationary,
    weights moving) so the final elementwise combine with the
    naturally-loaded h_prev and the output store need no transposes.
"""

import numpy as np

from concourse import bacc
import concourse.mybir as mybir
from concourse.tile import TileContext
from concourse.masks import make_identity
from concourse.bass_utils import run_bass_kernel_spmd

B, N, K, F, H = 64, 325, 3, 64, 1024
XW = K * F            # 192
CONCAT = XW + H       # 1216
NCORES = 8
TPC = (B * N) // NCORES   # 2600 tokens per core
TB = 256                  # token block size
F32 = mybir.dt.float32
MM_DT = mybir.dt.float32r  # matmul compute dtype (float32r: 4x faster PE)


def build(tpc=TPC, tb=TB, mm_dt=MM_DT):
    nc = bacc.Bacc("TRN2")
    x = nc.declare_dram_parameter("x", [tpc, XW], F32, isOutput=False)
    h = nc.declare_dram_parameter("h", [tpc, H], F32, isOutput=False)
    W = {}
    bvec = {}
    for g in "zrc":
        W[g] = nc.declare_dram_parameter(f"W{g}", [CONCAT, H], F32, isOutput=False)
        bvec[g] = nc.declare_dram_parameter(f"b{g}", [H], F32, isOutput=False)
    out = nc.declare_dram_parameter("out", [tpc, H], F32, isOutput=True)

    SIG = mybir.ActivationFunctionType.Sigmoid
    TANH = mybir.ActivationFunctionType.Tanh

    with TileContext(nc) as tc:
        with (
            tc.tile_pool(name="wpool", bufs=1) as wpool,
            tc.tile_pool(name="cpool", bufs=1) as cpool,
            tc.tile_pool(name="xnat", bufs=6) as xpool,
            tc.tile_pool(name="hnat", bufs=5) as hpool,
            tc.tile_pool(name="actT", bufs=20) as apool,
            tc.tile_pool(name="rh", bufs=8) as rhpool,
            tc.tile_pool(name="zsb", bufs=4) as zpool,
            tc.tile_pool(name="csb", bufs=3) as cbpool,
            tc.tile_pool(name="trps", bufs=2, space="PSUM") as trps,
            tc.tile_pool(name="rps", bufs=2, space="PSUM") as rps,
            tc.tile_pool(name="zcps", bufs=4, space="PSUM") as zcps,
        ):
            # Block table: full tb-token blocks, with the sub-128 remainder
            # merged into the final block (a tiny trailing block would pay
            # full LDWEIGHTS cost in its r GEMMs for almost no work).
            nblocks = tpc // tb
            rem = tpc - nblocks * tb
            btbs = [tb] * nblocks
            if rem >= 128 or nblocks == 0:
                btbs.append(rem)
            elif rem:
                btbs[-1] += rem
            blk = []
            t0 = 0
            for btb in btbs:
                blk.append((t0, btb))
                t0 += btb
            max_btb = max(b for _, b in blk)
            order = list(range(len(blk)))

            idt = cpool.tile([128, 128], F32, tag="idt")
            make_identity(nc, idt)
            ones = cpool.tile([1, max_btb], F32, tag="ones")
            nc.vector.memset(ones, 1.0)


            def emit_loads(bidx, chunked=False):
                t0, btb = blk[bidx]
                nsub = (btb + 127) // 128
                xts = []
                hts = []
                for s in range(nsub):
                    r0 = t0 + s * 128
                    ts_ = min(128, t0 + btb - r0)
                    # h before x: the transposes consume the h-part first
                    ht = hpool.tile([128, H], F32, tag="hnat", name=f"hn{bidx}_{s}")
                    if chunked:
                        # halve the first transfers so the first transposes
                        # can start ~1.5us earlier at kernel start
                        nc.sync.dma_start(
                            out=ht[:ts_, 0:512], in_=h[r0 : r0 + ts_, 0:512]
                        )
                        nc.sync.dma_start(
                            out=ht[:ts_, 512:H], in_=h[r0 : r0 + ts_, 512:H]
                        )
                    else:
                        nc.sync.dma_start(out=ht[:ts_, :], in_=h[r0 : r0 + ts_, :])
                    xt = xpool.tile([128, XW], F32, tag="xnat", name=f"xn{bidx}_{s}")
                    nc.sync.dma_start(out=xt[:ts_, :], in_=x[r0 : r0 + ts_, :])
                    xts.append((xt, ts_))
                    hts.append((ht, ts_))
                return xts, hts

            def emit_transposes(bidx, xts, hts):
                t0, btb = blk[bidx]
                nsub = (btb + 127) // 128
                a = [
                    apool.tile([128, max_btb], mm_dt, tag="actT", name=f"actT{bidx}_{i}")
                    for i in range(10)
                ]
                nc.scalar.copy(out=a[9][64:65, :btb], in_=ones[:, :btb])
                for s in range(nsub):
                    xt, ts_ = xts[s]
                    ht, _ = hts[s]
                    srcs = [(ht, 128 * k, 128, a[k]) for k in range(8)]
                    srcs += [(xt, 0, 128, a[8]), (xt, 128, 64, a[9])]
                    for src, c0, cw, dst in srcs:
                        pt = trps.tile([128, 128], F32, tag="trps")
                        nc.tensor.transpose(
                            pt[:cw, :ts_], src[:ts_, c0 : c0 + cw], idt[:ts_, :ts_]
                        )
                        nc.vector.tensor_copy(
                            out=dst[0:cw, s * 128 : s * 128 + ts_], in_=pt[:cw, :ts_]
                        )
                return a

            # Prologue: first two blocks' activations load + transpose before
            # any GEMM, so the PE has work while the 15MB of weights stream in.
            def emit_weights(g):
                # h-part tiles first (contraction runs h-first), then x-part
                # and the bias row folded into the final 65-row tile.
                tiles = []
                for k in range(8):
                    t = wpool.tile([128, H], mm_dt, tag=f"w{g}h{k}")
                    nc.sync.dma_start(
                        out=t,
                        in_=W[g][XW + 128 * k : XW + 128 * (k + 1), :].bitcast(mm_dt),
                    )
                    tiles.append((t, 128))
                t = wpool.tile([128, H], mm_dt, tag=f"w{g}0")
                nc.sync.dma_start(out=t, in_=W[g][0:128, :].bitcast(mm_dt))
                tiles.append((t, 128))
                t = wpool.tile([128, H], mm_dt, tag=f"w{g}1")
                nc.sync.dma_start(out=t[0:64, :], in_=W[g][128:192, :].bitcast(mm_dt))
                nc.sync.dma_start(
                    out=t[64:65, :], in_=bvec[g][:].unsqueeze(0).bitcast(mm_dt)
                )
                tiles.append((t, 65))
                return tiles

            # DMA priority order: first two blocks' activations, then the
            # weights in gate-use order (z warms the PE clock gate first).
            state = {}
            wt = {}
            state[order[0]] = emit_loads(order[0], chunked=True)
            if len(order) > 1:
                state[order[1]] = emit_loads(order[1], chunked=True)
            wt["z"] = emit_weights("z")
            wt["r"] = emit_weights("r")
            wt["c"] = emit_weights("c")

            acts = {}
            acts[order[0]] = emit_transposes(order[0], *state[order[0]])
            if len(order) > 1:
                acts[order[1]] = emit_transposes(order[1], *state[order[1]])

            def emit_r(bidx, a):
                btb = blk[bidx][1]
                rh = [
                    rhpool.tile([128, max_btb], mm_dt, tag="rh", name=f"rh{bidx}_{i}")
                    for i in range(8)
                ]
                for k in range(8):
                    pr = rps.tile([128, max_btb], F32, tag="rps")
                    for j, (wtile, kk) in enumerate(wt["r"]):
                        nc.tensor.matmul(
                            pr[:, :btb],
                            lhsT=wtile[:kk, 128 * k : 128 * (k + 1)],
                            rhs=a[j][:kk, :btb],
                            start=(j == 0),
                            stop=(j == 9),
                        )
                    nc.scalar.activation(out=rh[k][:, :btb], in_=pr[:, :btb], func=SIG)
                    nc.vector.tensor_mul(
                        rh[k][:, :btb], rh[k][:, :btb], a[k][:128, :btb]
                    )
                return rh

            def emit_z(bidx, a, xts):
                btb = blk[bidx][1]
                nsub = (btb + 127) // 128
                zts = []
                for s in range(nsub):
                    _, ts_ = xts[s]
                    zt = zpool.tile([128, H], F32, tag="zsb", name=f"z{bidx}_{s}")
                    for hh in range(2):
                        pz = zcps.tile([128, 512], F32, tag="zcps")
                        for j, (wtile, kk) in enumerate(wt["z"]):
                            nc.tensor.matmul(
                                pz[:ts_, :],
                                lhsT=a[j][:kk, s * 128 : s * 128 + ts_],
                                rhs=wtile[:kk, 512 * hh : 512 * (hh + 1)],
                                start=(j == 0),
                                stop=(j == 9),
                            )
                        nc.scalar.activation(
                            out=zt[:ts_, 512 * hh : 512 * (hh + 1)],
                            in_=pz[:ts_, :],
                            func=SIG,
                        )
                    zts.append(zt)
                return zts

            def emit_c(bidx, a, rh, zts, hts):
                t0, btb = blk[bidx]
                nsub = (btb + 127) // 128
                for s in range(nsub):
                    ht, ts_ = hts[s]
                    ct = cbpool.tile([128, H], F32, tag="csb", name=f"c{bidx}_{s}")
                    r0 = t0 + s * 128
                    for hh in range(2):
                        cs = slice(512 * hh, 512 * (hh + 1))
                        pc = zcps.tile([128, 512], F32, tag="zcps")
                        for j, (wtile, kk) in enumerate(wt["c"]):
                            lhs_src = rh[j] if j < 8 else a[j]
                            nc.tensor.matmul(
                                pc[:ts_, :],
                                lhsT=lhs_src[:kk, s * 128 : s * 128 + ts_],
                                rhs=wtile[:kk, cs],
                                start=(j == 0),
                                stop=(j == 9),
                            )
                        nc.scalar.activation(
                            out=ct[:ts_, cs], in_=pc[:ts_, :], func=TANH
                        )
                        # h_new = h + z*(c - h), in place in ct, per half so
                        # the DVE chain overlaps the other half's tanh
                        nc.vector.tensor_sub(
                            ct[:ts_, cs], ct[:ts_, cs], ht[:ts_, cs]
                        )
                        nc.vector.tensor_mul(
                            ct[:ts_, cs], ct[:ts_, cs], zts[s][:ts_, cs]
                        )
                        nc.vector.tensor_add(
                            ct[:ts_, cs], ct[:ts_, cs], ht[:ts_, cs]
                        )
                    nc.sync.dma_start(out=out[r0 : r0 + ts_, :], in_=ct[:ts_, :])

            if len(order) >= 2:
                # Startup interleave: z1 fills the PE while Wc still streams
                # in, so c0 never exposes a weight-arrival stall.
                b0, b1 = order[0], order[1]
                z0 = emit_z(b0, acts[b0], state[b0][0])
                rh0 = emit_r(b0, acts[b0])
                if len(order) > 2:
                    state[order[2]] = emit_loads(order[2])
                z1 = emit_z(b1, acts[b1], state[b1][0])
                emit_c(b0, acts[b0], rh0, z0, state[b0][1])
                if len(order) > 2:
                    acts[order[2]] = emit_transposes(order[2], *state[order[2]])
                if len(order) > 3:
                    state[order[3]] = emit_loads(order[3])
                rh1 = emit_r(b1, acts[b1])
                emit_c(b1, acts[b1], rh1, z1, state[b1][1])
                acts.pop(b0)
                acts.pop(b1)
                if len(order) > 3:
                    acts[order[3]] = emit_transposes(order[3], *state[order[3]])
                start_i = 2
            else:
                start_i = 0

            for i in range(start_i, len(order)):
                bidx = order[i]
                if i + 2 < len(order):
                    state[order[i + 2]] = emit_loads(order[i + 2])
                a = acts.pop(bidx)
                zts = emit_z(bidx, a, state[bidx][0])
                rh = emit_r(bidx, a)
                emit_c(bidx, a, rh, zts, state[bidx][1])
                if i + 2 < len(order):
                    acts[order[i + 2]] = emit_transposes(
                        order[i + 2], *state[order[i + 2]]
                    )

    nc.finalize()
    return nc


_NC_CACHE = {}


def _get_nc():
    key = (TPC, TB, str(MM_DT))
    if key not in _NC_CACHE:
        _NC_CACHE[key] = build()
    return _NC_CACHE[key]


def _make_in_maps(diffused_x, h_prev, Wz, bz, Wr, br, Wc, bc, tpc=TPC):
    x = np.ascontiguousarray(
        np.asarray(diffused_x, dtype=np.float32).reshape(B * N, XW)
    )
    hp = np.ascontiguousarray(np.asarray(h_prev, dtype=np.float32).reshape(B * N, H))
    shared = {
        "Wz": Wz, "bz": bz, "Wr": Wr, "br": br, "Wc": Wc, "bc": bc,
    }
    shared = {
        k: np.ascontiguousarray(np.asarray(v, dtype=np.float32))
        for k, v in shared.items()
    }
    in_maps = []
    for c in range(NCORES):
        sl = slice(c * tpc, (c + 1) * tpc)
        m = {"x": x[sl], "h": hp[sl]}
        m.update(shared)
        in_maps.append(m)
    return in_maps


def kernel(diffused_x, h_prev, Wz, bz, Wr, br, Wc, bc):
    nc = _get_nc()
    in_maps = _make_in_maps(diffused_x, h_prev, Wz, bz, Wr, br, Wc, bc)
    res = run_bass_kernel_spmd(nc, in_maps, list(range(NCORES)))
    outs = [res.results[c]["out"] for c in range(NCORES)]
    return np.concatenate(outs, axis=0).reshape(B, N, H)


def kernel_traced(diffused_x, h_prev, Wz, bz, Wr, br, Wc, bc):
    """Like kernel() but with NTFF profiling; returns (out, BassKernelResults)."""
    nc = _get_nc()
    in_maps = _make_in_maps(diffused_x, h_prev, Wz, bz, Wr, br, Wc, bc)
    res = run_bass_kernel_spmd(nc, in_maps, list(range(NCORES)), trace=True)
    outs = [res.results[c]["out"] for c in range(NCORES)]
    return np.concatenate(outs, axis=0).reshape(B, N, H), res
